# revision 32
# baseline (speedup 1.0000x reference)
#!/usr/bin/env python3
"""GroupedQueryAttention Trainium2 kernel, tensor-parallel over heads on 8
NeuronCores.

Reference model: B=2, S=2048, H=4096, NH=32 query heads, NKV=8 kv heads,
HD=128, RoPE base 5e5, softmax attention, o-proj.

Sharding: core c owns kv head c and query heads 4c..4c+3 (groups stay
aligned).  Wq/Wo sharded by query head, Wk/Wv by kv head.  Each core
computes a rank-512 slice of the o-proj contraction; the host sums the 8
partial outputs (the all-reduce of row-parallel TP done on host at gather
time).

v3 design notes:
  * All weights resident in SBUF in bf16, loaded once.  All matmuls bf16
    (full PE rate + FWL weight loads + halved DMA).
  * A and W phases fused: o-proj "units" (4 accumulating matmuls + evac)
    of earlier i-chunks are interleaved into the exp-gated score
    stretches, including across the batch boundary (the last i-chunk's
    o-proj of batch 0 fills the first i-chunk of batch 1's attention).
  * QKV projections evacuate PSUM through ScalarE to bf16 immediately;
    RoPE then runs as cheap 2x-mode bf16 DVE ops on SBUF, so the PSUM
    banks recycle fast and the P->A boundary only waits on a short chain.
  * kt/qt/ot are per-512-token-chunk tiles so Tile's dependency tracking
    never serializes readers on unrelated chunk writers.
  * Softmax denominators for the 4 heads of an i-chunk accumulate into
    one PSUM bank (one-hot column stationaries); a single bf16 DVE
    reciprocal covers all 4 heads and is issued a full iteration before
    the broadcast matmuls need it.
  * Output partials are written bf16 and summed on the host.
"""
import sys

for _p in ("/opt/trn_rl_repo",):
    if _p not in sys.path:
        sys.path.insert(0, _p)

import numpy as np

import concourse.bacc as bacc
import concourse.mybir as mybir
from concourse import tile
from concourse.bass_utils import run_bass_kernel_spmd

B, S, H = 2, 2048, 4096
NH, NKV, HD = 32, 8, 128
NCORES = 8
QH_PER_CORE = NH // NCORES          # 4 query heads / core
QD = QH_PER_CORE * HD               # 512 q dims / core
ROPE_BASE = 500000.0
T = B * S                           # 4096 tokens
TCH = 512                           # token chunk (proj N, attn i-chunk)
NTCH = S // TCH                     # 4 chunks per batch
HT = H // 128                       # 32 h-tiles
JT = S // 128                       # 16 j-tiles per batch
NHCH = H // TCH                     # 8 o-proj column chunks
SCALE = 1.0 / np.sqrt(HD)

F32 = mybir.dt.float32
F32R = mybir.dt.float32r
BF16 = mybir.dt.bfloat16


def _build_nc():
    nc = bacc.Bacc("TRN2", target_bir_lowering=False, debug=False)
    xt = nc.dram_tensor("xt", [H, T], BF16, kind="ExternalInput").ap()
    wq = nc.dram_tensor("wq", [H, QD], BF16, kind="ExternalInput").ap()
    wk = nc.dram_tensor("wk", [H, HD], BF16, kind="ExternalInput").ap()
    wv = nc.dram_tensor("wv", [H, HD], BF16, kind="ExternalInput").ap()
    wo = nc.dram_tensor("wo", [QD, H], BF16, kind="ExternalInput").ap()
    cosx = nc.dram_tensor("cosx", [HD, S], BF16, kind="ExternalInput").ap()
    ssin = nc.dram_tensor("ssin", [HD, S], BF16, kind="ExternalInput").ap()
    ident = nc.dram_tensor("ident", [128, 128], BF16, kind="ExternalInput").ap()
    # onescol[p, qh, j] = (j == qh): den matmul stationary per head
    onescol = nc.dram_tensor(
        "onescol", [128, QH_PER_CORE, QH_PER_CORE], BF16, kind="ExternalInput"
    ).ap()
    # bcsel[k, qh, m] = (k == qh): broadcast matmul stationary per head
    bcsel = nc.dram_tensor(
        "bcsel", [QH_PER_CORE, QH_PER_CORE, 128], BF16, kind="ExternalInput"
    ).ap()
    out = nc.dram_tensor("out_part", [T, H], BF16, kind="ExternalOutput").ap()

    with tile.TileContext(nc) as tc, \
         nc.allow_low_precision(reason="bf16 matmuls; bf16 attn probs"):
        with tc.tile_pool(name="persist", bufs=1) as persist, \
             tc.tile_pool(name="store", bufs=1) as spool:
            # ---- resident weights / tables ----
            # wq/wk/wv are chunked along HT (4 x 8 h-tiles) so the first
            # projection matmuls only wait on chunk 0; later chunks stream on
            # the sync queue interleaved with the first token-chunk's x tiles.
            # Non-critical tables go on the scalar queue in consumption order.
            HC = HT // 4
            wq_sb = [persist.tile([128, HC, QD], BF16, name=f"wq{c}")
                     for c in range(4)]
            wk_sb = [persist.tile([128, HC, HD], BF16, name=f"wk{c}")
                     for c in range(4)]
            wv_sb = [persist.tile([128, HC, HD], BF16, name=f"wv{c}")
                     for c in range(4)]

            def load_w_chunk(c, kv_only=False, q_only=False):
                r = slice(c * HC * 128, (c + 1) * HC * 128)
                if not q_only:
                    nc.sync.dma_start(
                        wk_sb[c][:],
                        wk[r, :].rearrange("(a p) q -> p a q", p=128))
                    nc.sync.dma_start(
                        wv_sb[c][:],
                        wv[r, :].rearrange("(a p) q -> p a q", p=128))
                if not kv_only:
                    nc.sync.dma_start(
                        wq_sb[c][:],
                        wq[r, :].rearrange("(a p) q -> p a q", p=128))

            load_w_chunk(0)
            cos_sb = persist.tile([HD, S], BF16)
            nc.scalar.dma_start(cos_sb[:], cosx[:])
            ssin_sb = persist.tile([HD, S], BF16)
            nc.scalar.dma_start(ssin_sb[:], ssin[:])
            ident_sb = persist.tile([128, 128], BF16)
            nc.scalar.dma_start(ident_sb[:], ident[:])
            onescol_sb = persist.tile([128, QH_PER_CORE, QH_PER_CORE], BF16)
            nc.scalar.dma_start(onescol_sb[:], onescol[:])
            bcsel_sb = persist.tile([QH_PER_CORE, QH_PER_CORE, 128], BF16)
            nc.scalar.dma_start(bcsel_sb[:], bcsel[:])
            wo_sb = persist.tile([128, QH_PER_CORE, H], BF16)
            nc.scalar.dma_start(wo_sb[:], wo.rearrange("(a p) h -> p a h", p=128))

            pend_w = []   # pending o-proj units, carried across batches
            cur = {}      # current batch's W pools/staging

            def emit_w_unit(unit):
                ots, ut0, ich, hch, tt = unit
                isl0 = ich * TCH
                key = (ut0, ich, hch)
                if key not in cur["oout"]:
                    cur["oout"][key] = cur["owp"].tile(
                        [128, 4, TCH], BF16, name="oout", tag="oout")
                w_ps = cur["wps"].tile([128, TCH], F32, name="wops", tag="wops")
                for od in range(QH_PER_CORE):
                    nc.tensor.matmul(
                        w_ps[:],
                        ots[od][ich][:, tt * 128:(tt + 1) * 128],
                        wo_sb[:, od, hch * TCH:(hch + 1) * TCH],
                        start=(od == 0), stop=(od == QH_PER_CORE - 1))
                ob = cur["oout"][key]
                nc.vector.tensor_copy(ob[:, tt, :], w_ps[:])
                if tt == 3:
                    nc.gpsimd.dma_start(
                        out[ut0 + isl0:ut0 + isl0 + TCH,
                            hch * TCH:(hch + 1) * TCH]
                        .rearrange("(a p) hh -> p a hh", p=128),
                        ob[:])
                    del cur["oout"][key]

            def drain_w(k):
                for _ in range(k):
                    if pend_w:
                        emit_w_unit(pend_w.pop(0))

            for b in range(B):
                t0 = b * S
                # per-batch activation stores, chunk-granular tiles
                qt_sb = [
                    [spool.tile([128, TCH], BF16, name=f"qt{qh}c{t}_b{b}",
                                tag=f"qt{qh}c{t}") for t in range(NTCH)]
                    for qh in range(QH_PER_CORE)
                ]
                kt_sb = [
                    spool.tile([128, TCH], BF16, name=f"ktc{t}_b{b}",
                               tag=f"ktc{t}") for t in range(NTCH)
                ]
                v_sb = spool.tile([128, JT, 128], BF16, name=f"v_b{b}", tag="v")
                ot_sb = [
                    [spool.tile([128, TCH], BF16, name=f"ot{qh}c{t}_b{b}",
                                tag=f"ot{qh}c{t}") for t in range(NTCH)]
                    for qh in range(QH_PER_CORE)
                ]

                # ---------------- P: QKV projections ----------------
                with tc.tile_pool(name="px", bufs=6) as px, \
                     tc.tile_pool(name="pt", bufs=2) as ptmp, \
                     tc.tile_pool(name="pps", bufs=1, space="PSUM") as pps, \
                     tc.tile_pool(name="vps", bufs=1, space="PSUM") as vps:
                    for tch in range(NTCH):
                        tc0 = t0 + tch * TCH
                        q_ps = [
                            pps.tile([128, TCH], F32, name=f"qps{i}", tag=f"qps{i}")
                            for i in range(QH_PER_CORE)
                        ]
                        k_ps = pps.tile([128, TCH], F32, name="kps", tag="kps")
                        v_ps = pps.tile([128, TCH], F32, name="vps0", tag="vps0")
                        for hg in range(HT // 2):
                            x_t = px.tile([128, 2, TCH], BF16, name="xs", tag="xs")
                            nc.sync.dma_start(
                                x_t[:],
                                xt[hg * 256:(hg + 1) * 256, tc0:tc0 + TCH]
                                .rearrange("(a p) t -> p a t", p=128))
                            if b == 0 and tch == 0 and hg % 4 == 0 and hg < 12:
                                load_w_chunk(hg // 4 + 1)
                            for hi in range(2):
                                h = hg * 2 + hi
                                hc, ho = divmod(h, HC)
                                first, last = h == 0, h == HT - 1
                                nc.tensor.matmul(k_ps[:], wk_sb[hc][:, ho, :],
                                                 x_t[:, hi, :], start=first,
                                                 stop=last)
                                nc.tensor.matmul(v_ps[:], wv_sb[hc][:, ho, :],
                                                 x_t[:, hi, :], start=first,
                                                 stop=last)
                                for qd in range(QH_PER_CORE):
                                    nc.tensor.matmul(
                                        q_ps[qd][:],
                                        wq_sb[hc][:, ho, qd * 128:(qd + 1) * 128],
                                        x_t[:, hi, :], start=first, stop=last)
                        # evacuate PSUM via ScalarE to bf16, RoPE on DVE in
                        # 2x bf16 mode.  K first (it gates the A phase).
                        csl = slice(tch * TCH, (tch + 1) * TCH)

                        # evac PSUM twice per rope head: as-is and with the
                        # halves partition-swapped (rotate_half; legal because
                        # the source is PSUM), so the DVE rope ops below are
                        # partition-aligned bf16 2x-mode.  All evacuations are
                        # emitted first, split across ScalarE and VectorE, so
                        # the PSUM banks free in ~half the serial-chain time
                        # and the next token-chunk's matmuls start sooner.
                        def evac(src_ps, raw_tag, eng):
                            raw = ptmp.tile([128, TCH], BF16, name=raw_tag,
                                            tag=raw_tag)
                            rsw = ptmp.tile([128, TCH], BF16,
                                            name=raw_tag + "s",
                                            tag=raw_tag + "s")
                            cp = nc.scalar.copy if eng == 0 else \
                                nc.vector.tensor_copy
                            cp(raw[:], src_ps[:])
                            cp(rsw[0:64, :], src_ps[64:128, :])
                            cp(rsw[64:128, :], src_ps[0:64, :])
                            return raw, rsw

                        def rope_tt(dst, raw, rsw):
                            tA = ptmp.tile([128, TCH], BF16, name="ropeA",
                                           tag="ropeA")
                            nc.vector.tensor_tensor(
                                tA[:], raw[:], cos_sb[:, csl],
                                mybir.AluOpType.mult)
                            tB = ptmp.tile([128, TCH], BF16, name="ropeB",
                                           tag="ropeB")
                            nc.vector.tensor_tensor(
                                tB[:], rsw[:], ssin_sb[:, csl],
                                mybir.AluOpType.mult)
                            nc.vector.tensor_tensor(
                                dst, tA[:], tB[:], mybir.AluOpType.add)

                        vraw = ptmp.tile([128, TCH], BF16, name="vraw",
                                         tag="vraw")
                        kr = evac(k_ps, "kraw", 0)
                        q0r = evac(q_ps[0], "q0raw", 1)
                        nc.scalar.copy(vraw[:], v_ps[:])
                        q1r = evac(q_ps[1], "q1raw", 0)
                        q2r = evac(q_ps[2], "q2raw", 1)
                        q3r = evac(q_ps[3], "q3raw", 0)
                        rope_tt(kt_sb[tch][:], *kr)
                        rope_tt(qt_sb[0][tch][:], *q0r)
                        rope_tt(qt_sb[1][tch][:], *q1r)
                        rope_tt(qt_sb[2][tch][:], *q2r)
                        rope_tt(qt_sb[3][tch][:], *q3r)
                        for tt in range(TCH // 128):
                            vt_ps = vps.tile([128, 128], BF16, name="vtp",
                                             tag="vtp")
                            nc.tensor.transpose(
                                vt_ps[:], vraw[:, tt * 128:(tt + 1) * 128],
                                ident_sb[:])
                            nc.vector.tensor_copy(
                                v_sb[:, tch * 4 + tt, :], vt_ps[:])

                # ---------------- A + W fused ----------------
                # PSUM pool creation order maps pools onto the banks the P
                # phase frees earliest: den/bc/PV land on q banks (needed a
                # full head later), score banks land on k/v (freed first).
                with tc.tile_pool(name="ap", bufs=2) as apool, \
                     tc.tile_pool(name="an", bufs=2) as anorm, \
                     tc.tile_pool(name="ow", bufs=2) as owp, \
                     tc.tile_pool(name="sps", bufs=3, space="PSUM") as sps, \
                     tc.tile_pool(name="ops", bufs=1, space="PSUM") as ops_, \
                     tc.tile_pool(name="dps", bufs=1, space="PSUM") as dps, \
                     tc.tile_pool(name="bps", bufs=1, space="PSUM") as bps, \
                     tc.tile_pool(name="wps", bufs=2, space="PSUM") as wps:
                    cur["owp"] = owp
                    cur["wps"] = wps
                    cur["oout"] = {}

                    den_ps = {}      # ich -> [4, TCH] PSUM tile
                    rec_sb = {}      # ich -> [4, TCH] bf16 reciprocal
                    ot_raw = {}      # (ich, qh) -> unnormalized PV output

                    def finish_prev(prev):
                        # evac PV output, in-place denominator tree over p_sb
                        # (PV already consumed it), den matmul; on the last
                        # head of an i-chunk also issue the reciprocal.
                        pich, pqh, p_sb, o_ps = prev
                        orw = apool.tile([128, TCH], BF16, name="oraw",
                                         tag=f"oraw{pqh}", bufs=1)
                        ot_raw[(pich, pqh)] = orw
                        nc.scalar.copy(orw[:], o_ps[:])
                        # denominator tree in-place over p_sb on DVE (GpSimd
                        # measured ~4x slower per elem -- not worth it)
                        nc.vector.tensor_tensor(
                            p_sb[:, 0:8, :], p_sb[:, 0:8, :], p_sb[:, 8:16, :],
                            mybir.AluOpType.add)
                        nc.vector.tensor_tensor(
                            p_sb[:, 0:4, :], p_sb[:, 0:4, :], p_sb[:, 4:8, :],
                            mybir.AluOpType.add)
                        nc.vector.tensor_tensor(
                            p_sb[:, 0:2, :], p_sb[:, 0:2, :], p_sb[:, 2:4, :],
                            mybir.AluOpType.add)
                        t1 = anorm.tile([128, TCH], BF16, name="t1", tag="t1",
                                        bufs=2)
                        nc.vector.tensor_tensor(
                            t1[:], p_sb[:, 0, :], p_sb[:, 1, :],
                            mybir.AluOpType.add)
                        if pich not in den_ps:
                            den_ps[pich] = dps.tile([QH_PER_CORE, TCH], F32,
                                                    name="den", tag="den")
                        nc.tensor.matmul(den_ps[pich][:], onescol_sb[:, pqh, :],
                                         t1[:], start=(pqh == 0),
                                         stop=(pqh == QH_PER_CORE - 1))
                        if pqh == QH_PER_CORE - 1:
                            rec = anorm.tile([QH_PER_CORE, TCH], BF16,
                                             name="rec", tag="rec", bufs=2)
                            nc.vector.reciprocal(rec[:], den_ps[pich][:])
                            rec_sb[pich] = rec
                            del den_ps[pich]

                    def emit_bc_norm(nich):
                        # per-head broadcast matmul + fused normalize-evac,
                        # then queue this i-chunk's o-proj units.
                        rec = rec_sb.pop(nich)
                        for qh in range(QH_PER_CORE):
                            bc_ps = bps.tile([128, TCH], F32, name="bc",
                                             tag="bc")
                            nc.tensor.matmul(bc_ps[:], bcsel_sb[:, qh, :],
                                             rec[:], start=True, stop=True)
                            nc.vector.tensor_tensor(
                                ot_sb[qh][nich][:], bc_ps[:],
                                ot_raw.pop((nich, qh))[:],
                                mybir.AluOpType.mult)
                        pend_w.extend(
                            (ot_sb, t0, nich, hch, tt)
                            for hch in range(NHCH) for tt in range(4))

                    prev = None
                    for n in range(NTCH * QH_PER_CORE):
                        ich, qh = divmod(n, QH_PER_CORE)
                        p_sb = apool.tile([128, JT, TCH], BF16, name="ptil",
                                          tag="ptil")
                        def sc(jt):
                            st_ps = sps.tile([128, TCH], F32, name="st",
                                             tag="st")
                            nc.tensor.matmul(
                                st_ps[:],
                                kt_sb[jt // 4][:, (jt % 4) * 128:
                                               (jt % 4 + 1) * 128],
                                qt_sb[qh][ich][:], start=True, stop=True)
                            nc.scalar.activation(
                                p_sb[:, jt, :], st_ps[:],
                                mybir.ActivationFunctionType.Exp,
                                scale=SCALE)

                        def pv(jt):
                            if prev is None:
                                return
                            _, _, pp_sb, po_ps = prev
                            nc.tensor.matmul(
                                po_ps[:], v_sb[:, jt, :], pp_sb[:, jt, :],
                                start=(jt == 0), stop=(jt == JT - 1))

                        for g in range(4):
                            jb = g * 4
                            sc(jb); sc(jb + 1)
                            pv(jb); pv(jb + 1)
                            drain_w(1)
                            sc(jb + 2); pv(jb + 2)
                            sc(jb + 3); pv(jb + 3)
                            drain_w(1)
                        if prev is not None:
                            finish_prev(prev)
                        if qh == 1 and ich >= 1:
                            emit_bc_norm(ich - 1)
                        o_ps_n = ops_.tile([128, TCH], F32, name="opv",
                                           tag="opv")
                        prev = (ich, qh, p_sb, o_ps_n)

                    # tail: PV + tree/den/reciprocal of the last head, final
                    # norm; o-proj units of the last i-chunk stay pending and
                    # fill the next batch's attention stalls (or drain fully
                    # at kernel end).
                    _, _, pp_sb, po_ps = prev
                    for jt in range(JT):
                        nc.tensor.matmul(po_ps[:], v_sb[:, jt, :],
                                         pp_sb[:, jt, :], start=(jt == 0),
                                         stop=(jt == JT - 1))
                        if jt % 4 == 3:
                            drain_w(2)
                    finish_prev(prev)
                    emit_bc_norm(NTCH - 1)
                    # drain half of the last i-chunk's o-proj in this batch's
                    # tail (it covers the denominator/normalize latency); the
                    # rest fills the next batch's first attention stalls.
                    drain_w(16 if b < B - 1 else len(pend_w))
    nc.finalize()
    return nc


_NC_CACHE = None


def _get_nc():
    global _NC_CACHE
    if _NC_CACHE is None:
        _NC_CACHE = _build_nc()
    return _NC_CACHE


def _host_tables():
    inv = 1.0 / (ROPE_BASE ** (np.arange(0, HD, 2, dtype=np.float64) / HD))
    t = np.arange(S, dtype=np.float64)
    freqs = np.outer(t, inv)                      # [S, 64]
    emb = np.concatenate([freqs, freqs], axis=1)  # [S, 128]
    cos = np.cos(emb).astype(np.float32).T.copy()   # [128, S]
    sin = np.sin(emb).astype(np.float32).T.copy()
    ssin = sin.copy()
    ssin[0:64, :] *= -1.0
    return np.ascontiguousarray(cos), np.ascontiguousarray(ssin)


def kernel(hidden_states, Wq, Wk, Wv, Wo, trace=False):
    import ml_dtypes
    BF = ml_dtypes.bfloat16

    hs = np.asarray(hidden_states, dtype=np.float32)
    Wq = np.asarray(Wq, dtype=np.float32)
    Wk = np.asarray(Wk, dtype=np.float32)
    Wv = np.asarray(Wv, dtype=np.float32)
    Wo = np.asarray(Wo, dtype=np.float32)

    xt = np.ascontiguousarray(hs.reshape(T, H).T).astype(BF)   # [H, T]
    cos, ssin = _host_tables()
    cos_bf = cos.astype(BF)
    ssin_bf = ssin.astype(BF)
    ident = np.eye(128, dtype=BF)
    onescol = np.zeros((128, QH_PER_CORE, QH_PER_CORE), dtype=BF)
    for qh in range(QH_PER_CORE):
        onescol[:, qh, qh] = 1
    bcsel = np.zeros((QH_PER_CORE, QH_PER_CORE, 128), dtype=BF)
    for qh in range(QH_PER_CORE):
        bcsel[qh, qh, :] = 1

    in_maps = []
    for c in range(NCORES):
        in_maps.append({
            "xt": xt,
            "wq": np.ascontiguousarray(Wq[c * QD:(c + 1) * QD, :].T).astype(BF),
            "wk": np.ascontiguousarray(Wk[c * HD:(c + 1) * HD, :].T).astype(BF),
            "wv": np.ascontiguousarray(Wv[c * HD:(c + 1) * HD, :].T).astype(BF),
            "wo": np.ascontiguousarray(Wo[:, c * QD:(c + 1) * QD].T).astype(BF),
            "cosx": cos_bf,
            "ssin": ssin_bf,
            "ident": ident,
            "onescol": onescol,
            "bcsel": bcsel,
        })

    nc = _get_nc()
    res = run_bass_kernel_spmd(nc, in_maps, list(range(NCORES)), trace=trace)
    acc = np.zeros((T, H), dtype=np.float32)
    for c in range(NCORES):
        acc += res.results[c]["out_part"].astype(np.float32)
    out = acc.reshape(B, S, H)
    if trace:
        return out, res
    return out
